# revision 7
# baseline (speedup 1.0000x reference)
"""Conv2d 3x3 (stride 1, pad 1) on Trainium2, data-parallel over batch.

Full problem: x [16, 32, 512, 512] f32, kernels [32, 32, 3, 3] f32
-> out [16, 32, 512, 512] f32.

Sharding: batch 16 / 8 cores = 2 images per core; kernels replicated.
No collectives needed.

Per-core kernel strategy (memory-bound target):
- Contraction over 96 partitions = 3 kh-groups x 32 in-channels, operands
  in float32r (same bytes as f32; the PE streams 1 output col/cycle at
  N >= 256 vs 4 cycles/col for exact f32, and fp32r precision is far
  inside the 2e-2 tolerance). Each kh-group holds a row-shifted copy of
  the zero-padded input rows, so output row m is 3 accumulating matmuls
  lhsT [96, 32] x rhs [96, 512] -> PSUM [32, 512], one per kw at rhs
  free offset m*514 + kw.
- float32r matmuls must target PSUM partition 0 with even N and 8B-
  aligned dst (4 PE cols per output col), so each output row owns a full
  PSUM bank [32, 512]; 8 banks rotate, kw-major order over 8-row groups
  amortizes weight loads. Evictions alternate Vector/Scalar engines.
- memset cannot target f32r tiles (ISA), so pad columns and image-edge
  halo rows are zeroed by Vector cast-copies from a small once-memset
  f32 zero tile.
- Input rows are read 3x from HBM (one DMA per kh-group): cheaper than
  SBUF->SBUF replication, which would pay read+write on the 435 GB/s
  AXI fabric; HBM (358 GB/s/core) has the headroom.
"""

import numpy as np
from contextlib import ExitStack

import concourse.bass as bass
import concourse.mybir as mybir
import concourse.tile as tile
from concourse.bass_utils import run_bass_kernel_spmd

F32 = mybir.dt.float32
F32R = mybir.dt.float32r
KH = KW = 3

# Full-problem geometry (hardcoded; kernel.py must be self-contained)
FULL_B, FULL_C, FULL_H, FULL_W = 16, 32, 512, 512
N_CORES = 8


def split_multi_waits(nc, cap=1):
    """This walrus build rejects instructions carrying more than `cap` sync
    wait commands ("Too many sync wait commands", setupSyncWait). Hoist
    excess waits onto single-wait NoOps inserted just before the instruction
    on the same engine queue (queues are in-order, so semantics are
    unchanged)."""
    n_split = 0
    for fn in nc.m.functions:
        for blk in fn.blocks:
            insts = blk.instructions
            if not any(
                i.sync_info is not None and len(i.sync_info.on_wait) > cap
                for i in insts
            ):
                continue
            new = []
            for inst in insts:
                si = inst.sync_info
                if si is not None and len(si.on_wait) > cap:
                    waits = list(si.on_wait)
                    n_split += 1
                    for k in range(0, len(waits) - cap, cap):
                        nop = mybir.InstNoOp(
                            name=nc.get_next_instruction_name(), ins=[], outs=[]
                        )
                        nop.engine = inst.engine
                        nop.sync_info = mybir.SyncInfo(
                            on_wait=waits[k : k + cap], on_update=[]
                        )
                        new.append(nop)
                    inst.sync_info = mybir.SyncInfo(
                        on_wait=waits[len(waits) - cap :],
                        on_update=list(si.on_update),
                    )
                new.append(inst)
            blk.instructions = new
    return n_split


def emit_conv(
    nc, tc, ctx, x_ap, w_ap, out_ap, B, C, H, W, M=16, prefix="", do_mm=True
):
    """x [B, C, H, W] local batch; w [3*C, 3*C] host-pretransposed
    (kh*C+ic, kw*C+oc); out [B, C, H, W]. M = output rows per block."""
    assert C == 32
    Wp = W + 2  # zero-padded row length
    T = H // M
    assert T * M == H
    GR = 8  # rows per kw-major matmul group (= PSUM banks in flight)

    w_pool = ctx.enter_context(tc.tile_pool(name=prefix + "wpool", bufs=1))
    z_pool = ctx.enter_context(tc.tile_pool(name=prefix + "zpool", bufs=1))
    in_pool = ctx.enter_context(tc.tile_pool(name=prefix + "inpool", bufs=2))
    out_pool = ctx.enter_context(tc.tile_pool(name=prefix + "outpool", bufs=2))
    psum_pool = ctx.enter_context(
        tc.tile_pool(name=prefix + "psumpool", bufs=8, space="PSUM")
    )

    w_tile = w_pool.tile([KH * C, KW * C], F32R, name=prefix + "w_tile", tag="w")
    nc.sync.dma_start(out=w_tile[:, :], in_=w_ap[:, :])
    # f32 zero tile: the only legal way to zero f32r SBUF is a cast-copy.
    z_tile = z_pool.tile([KH * C, Wp], F32, name=prefix + "z_tile", tag="z")
    nc.vector.memset(z_tile[:, :], 0.0)
    z_col = z_tile[:, 0:M].rearrange("p (s o) -> p s o", o=1)

    for b in range(B):
        for t in range(T):
            r0 = t * M
            # kh-group g slot s holds input row (r0-1+g+s): pad col 0 / Wp-1
            # zero, col 1+x <- x[b, :, row, x].
            in_tile = in_pool.tile(
                [KH * C, M * Wp], F32R, name=f"{prefix}in_{b}_{t}", tag="xin"
            )
            in_rows = in_tile.rearrange("p (s w) -> p s w", w=Wp)
            nc.vector.tensor_copy(in_rows[:, :, 0:1], z_col)
            nc.vector.tensor_copy(in_rows[:, :, Wp - 1 : Wp], z_col)
            for g in range(KH):
                lo = r0 - 1 + g
                hi = lo + M
                clo = max(lo, 0)
                chi = min(hi, H)
                s0 = clo - lo
                cnt = chi - clo
                dst = in_rows[32 * g : 32 * g + 32, s0 : s0 + cnt, 1 : 1 + W]
                nc.sync.dma_start(out=dst, in_=x_ap[b, :, clo:chi, :])
                if s0 > 0:  # top halo row (g=0, t=0): zero slot 0
                    nc.vector.tensor_copy(
                        in_rows[32 * g : 32 * g + 32, 0, :],
                        z_tile[32 * g : 32 * g + 32, :],
                    )
                if s0 + cnt < M:  # bottom halo row (g=2, t=T-1)
                    nc.vector.tensor_copy(
                        in_rows[32 * g : 32 * g + 32, M - 1, :],
                        z_tile[32 * g : 32 * g + 32, :],
                    )

            out_sb = out_pool.tile(
                [C, M * W], F32, name=f"{prefix}out_{b}_{t}", tag="osb"
            )
            if not do_mm:  # timing probe: mark tile written
                nc.vector.memset(out_sb[:, 0:1], 0.0)
            for m0 in range(0, M if do_mm else 0, GR):
                pts = [
                    psum_pool.tile(
                        [C, W], F32, name=f"{prefix}ps_{b}_{t}_{m0 + r}", tag="acc"
                    )
                    for r in range(GR)
                ]
                # kw-major: one weight load per kw per 8-row group
                for kw in range(KW):
                    lhsT = w_tile[:, kw * C : kw * C + C]
                    for r in range(GR):
                        fo = (m0 + r) * Wp + kw
                        nc.tensor.matmul(
                            pts[r][:, :],
                            lhsT,
                            in_tile[:, fo : fo + W],
                            start=(kw == 0),
                            stop=(kw == KW - 1),
                            tile_position=(0, 0),
                            skip_group_check=True,
                        )
                for r in range(GR):
                    m = m0 + r
                    dst = out_sb[:, m * W : (m + 1) * W]
                    if r % 2 == 0:
                        nc.vector.tensor_copy(dst, pts[r][:, :])
                    else:
                        nc.scalar.copy(dst, pts[r][:, :])

            # out_sb partition c, free m*W+x -> out[b, c, r0+m, x]
            src = out_sb.rearrange("c (s x) -> c s x", x=W)
            nc.sync.dma_start(out=out_ap[b, :, r0 : r0 + M, :], in_=src)


def build_conv_nc(B, C, H, W, M=16, do_mm=True):
    nc = bass.Bass("TRN2", target_bir_lowering=False, debug=False)
    x = nc.declare_dram_parameter("x", [B, C, H, W], F32R, isOutput=False)
    w = nc.declare_dram_parameter("kernels_t", [KH * C, KW * C], F32R, isOutput=False)
    out = nc.declare_dram_parameter("out", [B, C, H, W], F32, isOutput=True)
    with tile.TileContext(nc) as tc:
        with ExitStack() as ctx:
            emit_conv(nc, tc, ctx, x[:], w[:], out[:], B, C, H, W, M=M, do_mm=do_mm)
    split_multi_waits(nc, cap=1)
    return nc


_NC_CACHE = {}


def _get_nc():
    key = (FULL_B // N_CORES, FULL_C, FULL_H, FULL_W)
    if key not in _NC_CACHE:
        _NC_CACHE[key] = build_conv_nc(*key)
    return _NC_CACHE[key]


def host_weights(kernels: np.ndarray) -> np.ndarray:
    # [oc, ic, kh, kw] -> [(kh ic), (kw oc)] contiguous.
    return np.ascontiguousarray(
        kernels.transpose(2, 1, 3, 0).reshape(KH * 32, KW * 32)
    )


def kernel(x: np.ndarray, kernels: np.ndarray) -> np.ndarray:
    assert x.shape == (FULL_B, FULL_C, FULL_H, FULL_W), x.shape
    nc = _get_nc()
    bl = FULL_B // N_CORES
    wt = host_weights(np.asarray(kernels, dtype=np.float32))
    xs = np.asarray(x, dtype=np.float32)
    in_maps = [
        {"x": xs[i * bl : (i + 1) * bl], "kernels_t": wt} for i in range(N_CORES)
    ]
    res = run_bass_kernel_spmd(nc, in_maps, list(range(N_CORES))).results
    out = np.concatenate([res[i]["out"] for i in range(N_CORES)], axis=0)
    return out.astype(np.float32, copy=False)


# revision 11
# speedup vs baseline: 1.1543x; 1.1543x over previous
"""Conv2d 3x3 (stride 1, pad 1) on Trainium2, data-parallel over batch.

Full problem: x [16, 32, 512, 512] f32, kernels [32, 32, 3, 3] f32
-> out [16, 32, 512, 512] f32.

Sharding: batch 16 / 8 cores = 2 images per core; kernels replicated.
No collectives needed.

Per-core kernel strategy (memory-bound target):
- Contraction over 96 partitions = 3 kh-groups x 32 in-channels; operands
  bitcast to float32r (same bytes as f32; PE streams 1 output col/cycle
  at N >= 256 vs 4 cycles/col for exact f32; measured 142 ns/matmul).
  Output row m = 3 accumulating matmuls lhsT [96, 32] x rhs [96, ~512]
  -> PSUM [32, 512], one per kw tap.
- DMA descriptor efficiency dominates this part (measured ~30 us for a
  strided 1 MB DMA vs ~2.4 us contiguous): input rows are stored as raw
  512-col slots so every in-DMA is 32 descriptors of M*2KB contiguous
  bytes (rows of one channel are adjacent in HBM), and the out staging
  tile spans all 128 partitions (16 SDMA engines; 8 contiguous rows =
  16KB per partition-descriptor). No padded columns anywhere.
- kw=0 / kw=2 taps use even-N sub-range matmuls (fp32r requires even N
  and 8B-aligned PSUM dst): kw=0 -> dst [2:512), kw=2 -> dst [0:510),
  both reading img cols [1,511). The two missing single columns (x=1
  kw=0 tap = img col 0; x=510 kw=2 tap = img col 511) are computed by
  two tiny [96, 32] x [96, M]-strided fix-up matmuls per block and
  added into the staging tile after eviction.
- fp32r matmuls must write PSUM partition 0, so each output row owns a
  full PSUM bank [32, 512]; 8 banks rotate, kw-major order over 8-row
  groups amortizes weight loads. Evictions alternate Vector/Scalar;
  out-DMAs ride the scalar queue so the sync queue stays a pure
  in-DMA pipeline (no head-of-line blocking behind eviction waits).
- Input rows are read 3x from HBM (one DMA per kh-group): cheaper than
  SBUF->SBUF replication, which would pay read+write on the same SBUF
  AXI ports; HBM has the headroom.
"""

import numpy as np
from contextlib import ExitStack

import concourse.bass as bass
import concourse.mybir as mybir
import concourse.tile as tile
from concourse.bass_utils import run_bass_kernel_spmd

F32 = mybir.dt.float32
F32R = mybir.dt.float32r
KH = KW = 3

# Full-problem geometry (hardcoded; kernel.py must be self-contained)
FULL_B, FULL_C, FULL_H, FULL_W = 16, 32, 512, 512
N_CORES = 8


def split_multi_waits(nc, cap=1):
    """This walrus build rejects instructions carrying more than `cap` sync
    wait commands ("Too many sync wait commands", setupSyncWait). Hoist
    excess waits onto single-wait NoOps inserted just before the instruction
    on the same engine queue (queues are in-order, so semantics are
    unchanged)."""
    n_split = 0
    for fn in nc.m.functions:
        for blk in fn.blocks:
            insts = blk.instructions
            if not any(
                i.sync_info is not None and len(i.sync_info.on_wait) > cap
                for i in insts
            ):
                continue
            new = []
            for inst in insts:
                si = inst.sync_info
                if si is not None and len(si.on_wait) > cap:
                    waits = list(si.on_wait)
                    n_split += 1
                    for k in range(0, len(waits) - cap, cap):
                        nop = mybir.InstNoOp(
                            name=nc.get_next_instruction_name(), ins=[], outs=[]
                        )
                        nop.engine = inst.engine
                        nop.sync_info = mybir.SyncInfo(
                            on_wait=waits[k : k + cap], on_update=[]
                        )
                        new.append(nop)
                    inst.sync_info = mybir.SyncInfo(
                        on_wait=waits[len(waits) - cap :],
                        on_update=list(si.on_update),
                    )
                new.append(inst)
            blk.instructions = new
    return n_split


def emit_conv(
    nc, tc, ctx, x_ap, w_ap, out_ap, B, C, H, W, M=32, prefix="", do_mm=True
):
    """x [B, C, H, W] local batch; w [3*C, 3*C] host-pretransposed
    (kh*C+ic, kw*C+oc); out [B, C, H, W]. M = output rows per block."""
    assert C == 32
    T = H // M
    assert T * M == H
    GR = 8  # rows per kw-major matmul group (= PSUM banks in flight)
    JR = M // 4  # rows per out_sb partition group (4 groups of 32)

    w_pool = ctx.enter_context(tc.tile_pool(name=prefix + "wpool", bufs=1))
    in_pool = ctx.enter_context(tc.tile_pool(name=prefix + "inpool", bufs=2))
    out_pool = ctx.enter_context(tc.tile_pool(name=prefix + "outpool", bufs=2))
    psum_pool = ctx.enter_context(
        tc.tile_pool(name=prefix + "psumpool", bufs=8, space="PSUM")
    )

    w_tile = w_pool.tile([KH * C, KW * C], F32R, name=prefix + "w_tile", tag="w")
    nc.sync.dma_start(out=w_tile[:, :], in_=w_ap[:, :])
    # f32 zero tile: the only legal way to zero f32r SBUF is a cast-copy
    z_pool = ctx.enter_context(tc.tile_pool(name=prefix + "zpool", bufs=1))
    z_tile = z_pool.tile([KH * C, W], F32, name=prefix + "z_tile", tag="z")
    nc.vector.memset(z_tile[:, :], 0.0)

    def w_slice(kw):
        return w_tile[:, kw * C : kw * C + C]

    for b in range(B):
        for t in range(T):
            r0 = t * M
            # kh-group g slot s holds input row (r0-1+g+s), raw 512 cols.
            in_tile = in_pool.tile(
                [KH * C, M * W], F32R, name=f"{prefix}in_{b}_{t}", tag="xin"
            )
            in_rows = in_tile.rearrange("p (s w) -> p s w", w=W)
            for g in range(KH):
                lo = r0 - 1 + g
                hi = lo + M
                clo = max(lo, 0)
                chi = min(hi, H)
                s0 = clo - lo
                cnt = chi - clo
                # contiguous both sides: rows of one channel are adjacent
                dst = in_rows[32 * g : 32 * g + 32, s0 : s0 + cnt, :]
                nc.sync.dma_start(out=dst, in_=x_ap[b, :, clo:chi, :])
                if s0 > 0:  # top halo row (g=0, t=0): zero slot 0
                    nc.vector.tensor_copy(
                        in_rows[32 * g : 32 * g + 32, 0, :],
                        z_tile[32 * g : 32 * g + 32, :],
                    )
                if s0 + cnt < M:  # bottom halo row (g=2, t=T-1)
                    nc.vector.tensor_copy(
                        in_rows[32 * g : 32 * g + 32, M - 1, :],
                        z_tile[32 * g : 32 * g + 32, :],
                    )

            # staging: partition (j*32+c), free (s*W+x) -> row r0+JR*j+s
            out_sb = out_pool.tile(
                [128, JR * W], F32, name=f"{prefix}out_{b}_{t}", tag="osb"
            )
            if not do_mm:  # timing probe: mark tile written
                nc.vector.memset(out_sb[:, 0:1], 0.0)

            pts = {}
            for m0 in range(0, M if do_mm else 0, GR):
                grp = [
                    psum_pool.tile(
                        [C, W], F32, name=f"{prefix}ps_{b}_{t}_{m0 + r}", tag="acc"
                    )
                    for r in range(GR)
                ]
                for r in range(GR):
                    pts[m0 + r] = grp[r]
                # kw-major: one weight load per kw per 8-row group.
                # kw=1 covers all cols (start); kw=0 -> dst [2:512), kw=2 ->
                # dst [0:510) (fp32r: even N, 8B-aligned dst; the pad-tap
                # contribution is zero so those columns are simply skipped;
                # x=1/x=510 get their missing tap from the fix-up below).
                for kw in (1, 0, 2):
                    lhsT = w_slice(kw)
                    for r in range(GR):
                        base = (m0 + r) * W
                        if kw == 1:
                            dst = grp[r][:, :]
                            rhs = in_tile[:, base : base + W]
                        elif kw == 0:
                            dst = grp[r][:, 2:W]
                            rhs = in_tile[:, base + 1 : base + W - 1]
                        else:
                            dst = grp[r][:, 0 : W - 2]
                            rhs = in_tile[:, base + 1 : base + W - 1]
                        nc.tensor.matmul(
                            dst,
                            lhsT,
                            rhs,
                            start=(kw == 1),
                            stop=(kw == 2),
                            tile_position=(0, 0),
                            skip_group_check=True,
                        )
                for r in range(GR):
                    m = m0 + r
                    dst = out_sb[
                        32 * (m // JR) : 32 * (m // JR) + 32,
                        (m % JR) * W : (m % JR + 1) * W,
                    ]
                    if r % 2 == 0:
                        nc.vector.tensor_copy(dst, grp[r][:, :])
                    else:
                        nc.scalar.copy(dst, grp[r][:, :])

            if do_mm:
                # fix-up: col x=1 misses its kw=0 tap (img col 0); x=510
                # misses kw=2 (img col 511). One [96,32]x[96,M] matmul each
                # (rhs strided by a full row), added post-eviction.
                psfix = psum_pool.tile(
                    [C, 2 * M], F32, name=f"{prefix}psfix_{b}_{t}", tag="acc"
                )
                rhs_a = in_rows[:, :, 0:1]
                rhs_b = in_rows[:, :, W - 1 : W]
                nc.tensor.matmul(
                    psfix[:, 0:M], w_slice(0), rhs_a, start=True, stop=True,
                    tile_position=(0, 0), skip_group_check=True,
                )
                nc.tensor.matmul(
                    psfix[:, M : 2 * M], w_slice(2), rhs_b, start=True, stop=True,
                    tile_position=(0, 0), skip_group_check=True,
                )
                osb_rows = out_sb.rearrange("p (s x) -> p s x", x=W)
                for j in range(4):
                    for col, fo in ((1, 0), (W - 2, M)):
                        dst = osb_rows[
                            32 * j : 32 * j + 32, :, col : col + 1
                        ]
                        src = psfix[:, fo + JR * j : fo + JR * j + JR].rearrange(
                            "p (s o) -> p s o", o=1
                        )
                        nc.vector.tensor_add(dst, dst, src)

            # out_sb partition (j c), free (s x) -> out[b, c, r0+JR*j+s, x];
            # 4 DMAs over disjoint partition quarters engage all 16 SDMA
            # engines concurrently, each descriptor JR rows = 16KB contiguous.
            for j in range(4):
                src = out_sb[32 * j : 32 * j + 32, :].rearrange(
                    "c (s x) -> c s x", x=W
                )
                nc.scalar.dma_start(
                    out=out_ap[b, :, r0 + JR * j : r0 + JR * (j + 1), :], in_=src
                )


def build_conv_nc(B, C, H, W, M=32, do_mm=True):
    nc = bass.Bass("TRN2", target_bir_lowering=False, debug=False)
    x = nc.declare_dram_parameter("x", [B, C, H, W], F32R, isOutput=False)
    w = nc.declare_dram_parameter("kernels_t", [KH * C, KW * C], F32R, isOutput=False)
    out = nc.declare_dram_parameter("out", [B, C, H, W], F32, isOutput=True)
    with tile.TileContext(nc) as tc:
        with ExitStack() as ctx:
            emit_conv(nc, tc, ctx, x[:], w[:], out[:], B, C, H, W, M=M, do_mm=do_mm)
    split_multi_waits(nc, cap=1)
    return nc


_NC_CACHE = {}


def _get_nc():
    key = (FULL_B // N_CORES, FULL_C, FULL_H, FULL_W)
    if key not in _NC_CACHE:
        _NC_CACHE[key] = build_conv_nc(*key)
    return _NC_CACHE[key]


def host_weights(kernels: np.ndarray) -> np.ndarray:
    # [oc, ic, kh, kw] -> [(kh ic), (kw oc)] contiguous.
    return np.ascontiguousarray(
        kernels.transpose(2, 1, 3, 0).reshape(KH * 32, KW * 32)
    )


def kernel(x: np.ndarray, kernels: np.ndarray) -> np.ndarray:
    assert x.shape == (FULL_B, FULL_C, FULL_H, FULL_W), x.shape
    nc = _get_nc()
    bl = FULL_B // N_CORES
    wt = host_weights(np.asarray(kernels, dtype=np.float32))
    xs = np.asarray(x, dtype=np.float32)
    in_maps = [
        {"x": xs[i * bl : (i + 1) * bl], "kernels_t": wt} for i in range(N_CORES)
    ]
    res = run_bass_kernel_spmd(nc, in_maps, list(range(N_CORES))).results
    out = np.concatenate([res[i]["out"] for i in range(N_CORES)], axis=0)
    return out.astype(np.float32, copy=False)


# revision 13
# speedup vs baseline: 1.3527x; 1.1718x over previous
"""Conv2d 3x3 (stride 1, pad 1) on Trainium2, data-parallel over batch.

Full problem: x [16, 32, 512, 512] f32, kernels [32, 32, 3, 3] f32
-> out [16, 32, 512, 512] f32.

Sharding: batch 16 / 8 cores = 2 images per core; kernels replicated.
No collectives needed.

Per-core kernel strategy (memory-bound target):
- Inputs are converted to bf16 on the host (tolerance is 2e-2; bf16
  quantization contributes ~3e-3). This halves HBM input traffic and
  doubles PE column rate vs exact fp32 (which needs 4 cycles/col).
- Contraction over 96 partitions = 3 kh-groups x 32 in-channels. Each
  kh-group holds a row-shifted copy of the zero-padded input rows, so
  output row m is 3 accumulating matmuls lhsT [96, 32] x rhs [96, 512]
  -> PSUM, one per kw at rhs free offset m*514 + kw. 3x fewer, 3x wider
  matmuls than the per-tap 32x32 scheme (measured 241 ns/matmul).
- PSUM bank [128, 512] packs 4 output rows via matmul col groups
  (tile_position (0, 32j)); 12 matmuls fill a bank, 8 banks rotate per
  32-row block. Evictions are full-width [128, 512] copies alternating
  Vector/Scalar (narrow [32, 512] evictions measured ~6 us each and
  dominated an earlier fp32r variant, which cannot use col groups).
- Accumulation stays fp32 in PSUM; output is written as f32.
"""

import numpy as np
import ml_dtypes
from contextlib import ExitStack

import concourse.bass as bass
import concourse.mybir as mybir
import concourse.tile as tile
from concourse.bass_utils import run_bass_kernel_spmd

F32 = mybir.dt.float32
BF16 = mybir.dt.bfloat16
KH = KW = 3

# Full-problem geometry (hardcoded; kernel.py must be self-contained)
FULL_B, FULL_C, FULL_H, FULL_W = 16, 32, 512, 512
N_CORES = 8


def split_multi_waits(nc, cap=1):
    """This walrus build rejects instructions carrying more than `cap` sync
    wait commands ("Too many sync wait commands", setupSyncWait). Hoist
    excess waits onto single-wait NoOps inserted just before the instruction
    on the same engine queue (queues are in-order, so semantics are
    unchanged)."""
    n_split = 0
    for fn in nc.m.functions:
        for blk in fn.blocks:
            insts = blk.instructions
            if not any(
                i.sync_info is not None and len(i.sync_info.on_wait) > cap
                for i in insts
            ):
                continue
            new = []
            for inst in insts:
                si = inst.sync_info
                if si is not None and len(si.on_wait) > cap:
                    waits = list(si.on_wait)
                    n_split += 1
                    for k in range(0, len(waits) - cap, cap):
                        nop = mybir.InstNoOp(
                            name=nc.get_next_instruction_name(), ins=[], outs=[]
                        )
                        nop.engine = inst.engine
                        nop.sync_info = mybir.SyncInfo(
                            on_wait=waits[k : k + cap], on_update=[]
                        )
                        new.append(nop)
                    inst.sync_info = mybir.SyncInfo(
                        on_wait=waits[len(waits) - cap :],
                        on_update=list(si.on_update),
                    )
                new.append(inst)
            blk.instructions = new
    return n_split


def emit_conv(
    nc, tc, ctx, x_ap, w_ap, out_ap, B, C, H, W, M=32, prefix="", do_mm=True
):
    """x [B, C, H, W] local batch bf16; w [3*C, 3*C] host-pretransposed
    (kh*C+ic, kw*C+oc) bf16; out [B, C, H, W] f32. M = rows per block."""
    assert C == 32
    Wp = W + 2  # zero-padded row length
    T = H // M
    assert T * M == H
    QB = M // 4  # PSUM banks per block (4 output rows each)
    assert QB <= 8

    w_pool = ctx.enter_context(tc.tile_pool(name=prefix + "wpool", bufs=1))
    in_pool = ctx.enter_context(tc.tile_pool(name=prefix + "inpool", bufs=3))
    out_pool = ctx.enter_context(tc.tile_pool(name=prefix + "outpool", bufs=2))
    psum_pool = ctx.enter_context(
        tc.tile_pool(name=prefix + "psumpool", bufs=8, space="PSUM")
    )

    w_tile = w_pool.tile([KH * C, KW * C], BF16, name=prefix + "w_tile", tag="w")
    nc.sync.dma_start(out=w_tile[:, :], in_=w_ap[:, :])

    for b in range(B):
        for t in range(T):
            r0 = t * M
            # kh-group g slot s holds input row (r0-1+g+s): pad cols 0 /
            # Wp-1 zero, col 1+x <- x[b, :, row, x].
            in_tile = in_pool.tile(
                [KH * C, M * Wp], BF16, name=f"{prefix}in_{b}_{t}", tag="xin"
            )
            in_rows = in_tile.rearrange("p (s w) -> p s w", w=Wp)
            nc.vector.memset(in_rows[:, :, 0:1], 0.0)
            nc.vector.memset(in_rows[:, :, Wp - 1 : Wp], 0.0)
            for g in range(KH):
                lo = r0 - 1 + g
                hi = lo + M
                clo = max(lo, 0)
                chi = min(hi, H)
                s0 = clo - lo
                cnt = chi - clo
                dst = in_rows[32 * g : 32 * g + 32, s0 : s0 + cnt, 1 : 1 + W]
                nc.sync.dma_start(out=dst, in_=x_ap[b, :, clo:chi, :])
                if s0 > 0:  # top halo row (g=0, t=0): zero slot 0
                    nc.vector.memset(in_rows[32 * g : 32 * g + 32, 0:s0, :], 0.0)
                if s0 + cnt < M:  # bottom halo row (g=2, t=T-1)
                    nc.vector.memset(
                        in_rows[32 * g : 32 * g + 32, s0 + cnt : M, :], 0.0
                    )

            # staging: partition (j*32+c), free (q*W+x) -> row r0+4q+j
            out_sb = out_pool.tile(
                [128, QB * W], F32, name=f"{prefix}out_{b}_{t}", tag="osb"
            )
            if not do_mm:  # timing probe: mark tile written
                nc.vector.memset(out_sb[:, 0:1], 0.0)
            for q in range(QB if do_mm else 0):
                pt = psum_pool.tile(
                    [128, W], F32, name=f"{prefix}ps_{b}_{t}_{q}", tag="acc"
                )
                # kw-major: one weight load per kw per bank; col group j
                # holds output row 4q+j.
                for kw in range(KW):
                    lhsT = w_tile[:, kw * C : kw * C + C]
                    for j in range(4):
                        fo = (4 * q + j) * Wp + kw
                        nc.tensor.matmul(
                            pt[32 * j : 32 * j + 32, :],
                            lhsT,
                            in_tile[:, fo : fo + W],
                            start=(kw == 0),
                            stop=(kw == KW - 1),
                            tile_position=(0, 32 * j),
                            # 4 col groups share the bank (disjoint partition
                            # slices); the sim's group check is
                            # partition-coarse and false-positives.
                            skip_group_check=True,
                        )
                dst = out_sb[:, q * W : (q + 1) * W]
                if q % 2 == 0:
                    nc.vector.tensor_copy(dst, pt[:, :])
                else:
                    nc.scalar.copy(dst, pt[:, :])

            # out_sb partition 32j+c, free q*W+x -> out[b, c, r0+4q+j, x]
            dstv = out_ap[b, :, r0 : r0 + M, :].rearrange(
                "c (q j) x -> j c q x", j=4
            )
            for j in range(4):
                src = out_sb[32 * j : 32 * j + 32, :].rearrange(
                    "c (q x) -> c q x", x=W
                )
                nc.sync.dma_start(out=dstv[j], in_=src)


def build_conv_nc(B, C, H, W, M=32, do_mm=True):
    nc = bass.Bass("TRN2", target_bir_lowering=False, debug=False)
    x = nc.declare_dram_parameter("x", [B, C, H, W], BF16, isOutput=False)
    w = nc.declare_dram_parameter("kernels_t", [KH * C, KW * C], BF16, isOutput=False)
    out = nc.declare_dram_parameter("out", [B, C, H, W], F32, isOutput=True)
    with tile.TileContext(nc) as tc:
        with ExitStack() as ctx:
            emit_conv(nc, tc, ctx, x[:], w[:], out[:], B, C, H, W, M=M, do_mm=do_mm)
    split_multi_waits(nc, cap=1)
    return nc


_NC_CACHE = {}


def _get_nc():
    key = (FULL_B // N_CORES, FULL_C, FULL_H, FULL_W)
    if key not in _NC_CACHE:
        _NC_CACHE[key] = build_conv_nc(*key)
    return _NC_CACHE[key]


def host_weights(kernels: np.ndarray) -> np.ndarray:
    # [oc, ic, kh, kw] -> [(kh ic), (kw oc)] contiguous bf16.
    return np.ascontiguousarray(
        kernels.transpose(2, 1, 3, 0).reshape(KH * 32, KW * 32)
    ).astype(ml_dtypes.bfloat16)


def host_x(x: np.ndarray) -> np.ndarray:
    return np.ascontiguousarray(x).astype(ml_dtypes.bfloat16)


def kernel(x: np.ndarray, kernels: np.ndarray) -> np.ndarray:
    assert x.shape == (FULL_B, FULL_C, FULL_H, FULL_W), x.shape
    nc = _get_nc()
    bl = FULL_B // N_CORES
    wt = host_weights(np.asarray(kernels, dtype=np.float32))
    xs = host_x(np.asarray(x, dtype=np.float32))
    in_maps = [
        {"x": xs[i * bl : (i + 1) * bl], "kernels_t": wt} for i in range(N_CORES)
    ]
    res = run_bass_kernel_spmd(nc, in_maps, list(range(N_CORES))).results
    out = np.concatenate([res[i]["out"] for i in range(N_CORES)], axis=0)
    return out.astype(np.float32, copy=False)


# revision 14
# speedup vs baseline: 1.4089x; 1.0416x over previous
"""Conv2d 3x3 (stride 1, pad 1) on Trainium2, data-parallel over batch.

Full problem: x [16, 32, 512, 512] f32, kernels [32, 32, 3, 3] f32
-> out [16, 32, 512, 512] f32.

Sharding: batch 16 / 8 cores = 2 images per core; kernels replicated.
No collectives needed.

Per-core kernel strategy (memory-bound target):
- Inputs are converted to bf16 on the host (tolerance is 2e-2; bf16
  quantization contributes ~2e-3): halves HBM input traffic and runs the
  PE at 1 cycle/col instead of fp32's 4 (2 half-speed passes).
- Conv expressed as 9 accumulating 32x32 matmuls (one per tap) into PSUM.
- The 128x128 PE array is addressed as 16 concurrent 32x32 sub-arrays via
  tile_position: row group i = image band i (4 horizontal bands), col
  group j = output-row slot j (4 rows in flight per band). This 4x4
  tiling overlaps the sub-matmuls; wider (96-row) matmul shapes measured
  4-6x slower here despite fewer instructions.
- Input rows live in SBUF as [32 ch, (R+2) x (W+2)] bf16 with zero-padded
  columns, so every tap (kh, kw) is just a free-dim offset: kh*514 + kw.
- PSUM tile [128, 512] f32 per (band, step) holds 4 output rows (one per
  col group); evicted to SBUF by full-width Vector/Scalar copies (narrow
  [32, 512] evictions measured ~6 us each in an earlier variant), DMA'd
  out with a scatter access pattern.
"""

import numpy as np
import ml_dtypes
from contextlib import ExitStack

import concourse.bass as bass
import concourse.mybir as mybir
import concourse.tile as tile
from concourse.bass_utils import run_bass_kernel_spmd

F32 = mybir.dt.float32
BF16 = mybir.dt.bfloat16
KH = KW = 3
NBANDS = 4  # row groups = horizontal image bands
NCOLG = 4  # col groups = output rows in flight per band

# Full-problem geometry (hardcoded; kernel.py must be self-contained)
FULL_B, FULL_C, FULL_H, FULL_W = 16, 32, 512, 512
N_CORES = 8


def split_multi_waits(nc, cap=1):
    """This walrus build rejects instructions carrying more than `cap` sync
    wait commands ("Too many sync wait commands", setupSyncWait). Hoist
    excess waits onto single-wait NoOps inserted just before the instruction
    on the same engine queue (queues are in-order, so semantics are
    unchanged)."""
    n_split = 0
    for fn in nc.m.functions:
        for blk in fn.blocks:
            insts = blk.instructions
            if not any(
                i.sync_info is not None and len(i.sync_info.on_wait) > cap
                for i in insts
            ):
                continue
            new = []
            for inst in insts:
                si = inst.sync_info
                if si is not None and len(si.on_wait) > cap:
                    waits = list(si.on_wait)
                    n_split += 1
                    for k in range(0, len(waits) - cap, cap):
                        nop = mybir.InstNoOp(
                            name=nc.get_next_instruction_name(), ins=[], outs=[]
                        )
                        nop.engine = inst.engine
                        nop.sync_info = mybir.SyncInfo(
                            on_wait=waits[k : k + cap], on_update=[]
                        )
                        new.append(nop)
                    inst.sync_info = mybir.SyncInfo(
                        on_wait=waits[len(waits) - cap :],
                        on_update=list(si.on_update),
                    )
                new.append(inst)
            blk.instructions = new
    return n_split


def emit_conv(
    nc, tc, ctx, x_ap, w_ap, out_ap, B, C, H, W, R=16, prefix="", do_mm=True
):
    """Emit the Tile program for a per-core conv: x [B, C, H, W] (local
    batch, bf16), w [C, KH*KW*C] (pre-transposed on host: [ic, (kh kw oc)],
    bf16), out [B, C, H, W] f32."""
    assert C == 32
    HB = H // NBANDS  # rows per band
    assert HB * NBANDS == H
    assert HB % R == 0
    T = HB // R  # rounds per image
    S = R // NCOLG  # steps per round (4 rows per step per band)
    assert S * NCOLG == R
    Wp = W + 2  # zero-padded row length
    assert W <= 512

    w_pool = ctx.enter_context(tc.tile_pool(name=prefix + "wpool", bufs=1))
    in_pool = ctx.enter_context(tc.tile_pool(name=prefix + "inpool", bufs=3))
    out_pool = ctx.enter_context(tc.tile_pool(name=prefix + "outpool", bufs=2 * NBANDS))
    psum_pool = ctx.enter_context(
        tc.tile_pool(name=prefix + "psumpool", bufs=2 * NBANDS, space="PSUM")
    )

    # Weights: replicate [32, 9*32] into each of the 4 partition groups so
    # lhsT.base_partition() matches the rhs row group.
    w_tile = w_pool.tile([128, KH * KW * C], BF16, name=prefix + "w_tile", tag="w")
    for r in range(NBANDS):
        nc.sync.dma_start(out=w_tile[32 * r : 32 * r + 32, :], in_=w_ap[:, :])

    for b in range(B):
        for t in range(T):
            # ---- load input rows for this round: band i covers output rows
            # [i*HB + t*R, i*HB + t*R + R), needing input rows -1..R+1 around it.
            in_tile = in_pool.tile(
                [128, (R + 2) * Wp], BF16, name=f"{prefix}in_{b}_{t}", tag="xin"
            )
            in_rows = in_tile.rearrange("p (r w) -> p r w", w=Wp)
            # zero the left/right pad columns for all row slots
            nc.vector.memset(in_rows[:, :, 0:1], 0.0)
            nc.vector.memset(in_rows[:, :, Wp - 1 : Wp], 0.0)
            for i in range(NBANDS):
                row0 = i * HB + t * R
                lo = max(row0 - 1, 0)
                hi = min(row0 + R + 1, H)
                slot0 = lo - (row0 - 1)
                cnt = hi - lo
                dst = in_rows[32 * i : 32 * i + 32, slot0 : slot0 + cnt, 1 : 1 + W]
                nc.sync.dma_start(out=dst, in_=x_ap[b, :, lo:hi, :])
                if row0 == 0:  # top image boundary: zero row slot 0
                    nc.vector.memset(in_rows[32 * i : 32 * i + 32, 0:1, :], 0.0)
                if row0 + R == H:  # bottom image boundary: zero last slot
                    nc.vector.memset(
                        in_rows[32 * i : 32 * i + 32, R + 1 : R + 2, :], 0.0
                    )

            out_tiles = []
            for i in range(NBANDS):
                ot = out_pool.tile(
                    [128, S * W], F32, name=f"{prefix}out_{b}_{t}_{i}", tag="osb"
                )
                if not do_mm:  # timing probe: mark tile written
                    nc.vector.memset(ot[:, 0:1], 0.0)
                out_tiles.append(ot)

            SG = 1  # steps sharing one weight load (2 broke tile scheduling)
            for sg in range(0, S if do_mm else 0, SG):
                psums = {}
                for s2 in range(SG):
                    for i in range(NBANDS):
                        pt = psum_pool.tile(
                            [128, W],
                            F32,
                            name=f"{prefix}ps_{b}_{t}_{sg + s2}_{i}",
                            tag="acc",
                        )
                        psums[(s2, i)] = pt
                # 9 taps; 16 concurrent 32x32 sub-array matmuls per tap; each
                # sub-array runs SG rows back-to-back on one weight load
                for off in range(KH * KW):
                    kh, kw = off // KW, off % KW
                    for i in range(NBANDS):
                        lhsT = w_tile[32 * i : 32 * i + 32, off * C : off * C + C]
                        for j in range(NCOLG):
                            for s2 in range(SG):
                                m = NCOLG * (sg + s2) + j  # local output row
                                fo = (m + kh) * Wp + kw
                                rhs = in_tile[32 * i : 32 * i + 32, fo : fo + W]
                                nc.tensor.matmul(
                                    psums[(s2, i)][32 * j : 32 * j + 32, :],
                                    lhsT,
                                    rhs,
                                    start=(off == 0),
                                    stop=(off == KH * KW - 1),
                                    tile_position=(32 * i, 32 * j),
                                    # 4 col groups share each bank (disjoint
                                    # partition slices); the sim's group check
                                    # is partition-coarse and false-positives.
                                    skip_group_check=True,
                                )
                # evict: one [128, W] copy per band per step (4 rows each)
                for s2 in range(SG):
                    for i in range(NBANDS):
                        dst = out_tiles[i][:, (sg + s2) * W : (sg + s2 + 1) * W]
                        if i % 2 == 0:
                            nc.vector.tensor_copy(dst, psums[(s2, i)][:, :])
                        else:
                            nc.scalar.copy(dst, psums[(s2, i)][:, :])

            # ---- store: out_tile [128, S*W] partition 32j+c, free s*W+x
            # maps to out[b, c, row0 + 4s + j, x]
            for i in range(NBANDS):
                row0 = i * HB + t * R
                dstv = out_ap[b, :, row0 : row0 + R, :].rearrange(
                    "c (s j) x -> j c s x", s=S, j=NCOLG
                )
                for j in range(NCOLG):
                    src = out_tiles[i][32 * j : 32 * j + 32, :].rearrange(
                        "c (s x) -> c s x", x=W
                    )
                    nc.sync.dma_start(out=dstv[j], in_=src)


def build_conv_nc(B, C, H, W, R=16, passes=1, do_mm=True):
    nc = bass.Bass("TRN2", target_bir_lowering=False, debug=False)
    x = nc.declare_dram_parameter("x", [B, C, H, W], BF16, isOutput=False)
    w = nc.declare_dram_parameter("kernels_t", [C, KH * KW * C], BF16, isOutput=False)
    out = nc.declare_dram_parameter("out", [B, C, H, W], F32, isOutput=True)
    with tile.TileContext(nc) as tc:
        with ExitStack() as ctx:
            emit_conv(nc, tc, ctx, x[:], w[:], out[:], B, C, H, W, R=R, do_mm=do_mm)
    split_multi_waits(nc, cap=1)
    return nc


_NC_CACHE = {}


def _get_nc():
    key = (FULL_B // N_CORES, FULL_C, FULL_H, FULL_W)
    if key not in _NC_CACHE:
        _NC_CACHE[key] = build_conv_nc(*key)
    return _NC_CACHE[key]


def host_weights(kernels: np.ndarray) -> np.ndarray:
    # [oc, ic, kh, kw] -> [ic, (kh kw oc)] contiguous bf16, so the weight
    # DMA is a plain 2D copy.
    return np.ascontiguousarray(
        kernels.transpose(1, 2, 3, 0).reshape(32, -1)
    ).astype(ml_dtypes.bfloat16)


def host_x(x: np.ndarray) -> np.ndarray:
    return np.ascontiguousarray(x).astype(ml_dtypes.bfloat16)


def kernel(x: np.ndarray, kernels: np.ndarray) -> np.ndarray:
    assert x.shape == (FULL_B, FULL_C, FULL_H, FULL_W), x.shape
    nc = _get_nc()
    bl = FULL_B // N_CORES
    wt = host_weights(np.asarray(kernels, dtype=np.float32))
    xs = host_x(np.asarray(x, dtype=np.float32))
    in_maps = [
        {"x": xs[i * bl : (i + 1) * bl], "kernels_t": wt} for i in range(N_CORES)
    ]
    res = run_bass_kernel_spmd(nc, in_maps, list(range(N_CORES))).results
    out = np.concatenate([res[i]["out"] for i in range(N_CORES)], axis=0)
    return out.astype(np.float32, copy=False)
